# revision 32
# baseline (speedup 1.0000x reference)
"""Trainium2 Bass kernel for the ConvolutionalKAN problem.

Math: the KAN conv
    out[b,o,y,x] = sum_{j,kk,l,m} phi_m(11*inp[b,j,y+kk,x+l]) * coeff[o,j,kk,l,m]
with phi_m the degree-3 B-spline basis on uniform knots linspace(0,1,12).
Uniform knots -> phi_m(t) = N3(t-m) with N3 the cardinal cubic B-spline:
    6*N3 = a^3 - 4*b^3,  a = relu(2-u), b = relu(1-u) = relu(a-1), u = |t-(m+2)|
Weights fold to coeff/6 exactly, making this a VALID 3x3 conv over
64*8 = 512 input channels.

v3 design (vs the 146us v2, which was PE-bound at 50% column utilization):
- Tap pairing: the 6 taps with l in {0,1} ride 128-wide lhsTs = two taps
  ((kk,0)|(kk,1)) x 64 Cout sharing one [nr,63] rhs stream, so all 128 PE
  output columns are used for 2/3 of the work.  The 3 l=2 taps stay
  64-wide but fold into the SAME psum bank: their shifted [nr,62] windows
  plus a strided out-AP land them exactly on the anchor (l=0) alignment.
  One accumulation bank per 8-row output group:
      psA[0:64,  dy, x]   = sum of taps (kk,0) and (kk,2) partials
      psA[64:128, dy, x+1] = sum of taps (kk,1) partials
  PE cycles drop from ~287k to ~186k per core; the combine is only
  1 ACT copy (Scalar, psum->sbuf) + 1 shifted add (Vector, sbuf+psum)
  per group.
- Basis eval is split: most (q,strip) units compute the spline window
  a = relu(2-|11x-(m+2)|) on the Scalar engine (Abs+Relu ACTs); q3 units
  use the custom DVE op KAN_WIN on Vector.  The cube
  g = a^3 - 4*relu(a-1)^3 always runs on Vector (custom DVE op KAN_CUBE,
  bf16 out).
- Startup: weights ride the sync HW-DGE queue immediately; image 0's
  first 10 input rows form a tiny sub-strip so the first matmul chain
  starts early.  All 8 psum banks cycle through the 8-group pipeline.

Sharding: data-parallel over batch, 2 images per core on 8 cores.
"""

import os
import sys

import numpy as np

for _p in ("/root/.axon_site/_ro/trn_rl_repo", "/opt/trn_rl_repo"):
    if os.path.isdir(_p) and _p not in sys.path:
        sys.path.append(_p)

B_FULL = 16
N_CORES = 8
B_SHARD = B_FULL // N_CORES
CIN = 64
COUT = 64
H = 64
W = 64
KS = 3
NB = 8
NS = 8
HO = H - KS + 1  # 62
WO = W - KS + 1  # 62
NQ = (CIN * NS) // 128  # 4 contraction tiles of 128
NRG = 8  # output rows per group
NG = 8  # groups per image (7x8 + 1x6)

# weight column layout (per 128-row contraction chunk):
#   cols kk*128+[0:64]  = tap (kk,0), kk*128+[64:128] = tap (kk,1)
#   cols 384+kk*128+[0:64] = tap (kk,2), [64:128] zero (pad keeps every
#   LDWEIGHTS 128 wide; the zero half accumulates 0 into psA[64:128])
WCOLS = 768

# strips: (name, first input row, n rows); group g (out rows 8g..8g+nr-1)
# reads input rows 8g..8g+nr+1.  Image 0 splits the first strip so group
# 0's basis is ready early.
STRIPS_IMG0 = (("s0a", 0, 10), ("s0b1", 8, 10), ("s0b2", 16, 10),
               ("s1a", 24, 10), ("s1b", 32, 10), ("s1c", 40, 10),
               ("s2", 48, 16))
G2S_IMG0 = ("s0a", "s0b1", "s0b2", "s1a", "s1b", "s1c", "s2", "s2")
STRIPS_IMG1 = (("sA", 0, 26), ("sB", 24, 26), ("sC", 48, 16))
G2S_IMG1 = ("sA", "sA", "sA", "sB", "sB", "sB", "sC", "sC")

UMAX = 26 * W  # largest strip, scratch tile size

_DVE_OP_CACHE = {}


def _register_dve_op(name, spec):
    from concourse import dve_ops
    from concourse.dve_spec import lower
    from concourse.dve_uop import DveOpSpec
    from concourse.dve_spec import _has_src1

    existing = {op.name for op in dve_ops.OPS}
    if name in existing:
        return next(o for o in dve_ops.OPS if o.name == name)
    row = dve_ops._CUSTOM_DVE_ROW_BASE + len(dve_ops.OPS)
    shas = {}
    for ver in ("v3", "v4"):
        s = DveOpSpec(name=name, opcode=row, uops=lower(spec, ver=ver),
                      rd1_en=_has_src1(spec))
        shas[ver] = s.sha(ver)
    op = dve_ops.DveOp(name, spec, subdim=False, uops_sha=shas)
    dve_ops.OPS.append(op)
    dve_ops._SUB_OPCODE_FOR_NAME[name] = row
    return op


def _get_kan_ops():
    """Register (once) and return the two custom DVE ops:
    KAN_WIN:  a = relu(min(s0 - 11*x, 11*x - s1))  (= relu(2-|11x-(m+2)|)
              for s0 = m+4, s1 = m)
    KAN_CUBE: g = a^3 - 4*relu(a-1)^3              (= 6*N3(|11x-(m+2)|))
    """
    if "ops" in _DVE_OP_CACHE:
        return _DVE_OP_CACHE["ops"]
    from concourse.dve_spec import C0, C1, C2, One, Spec, Src0, minn, relu, sq

    m = Src0 * C2
    win_spec = Spec(
        body=relu(minn(C0 - m, m - C1)),
        reference=lambda in0, in1, s0, s1, imm2: np.maximum(
            np.minimum(s0 - in0 * imm2, in0 * imm2 - s1), 0.0
        ).astype(np.float32),
    )
    a = Src0
    b = relu(a - One)
    cube_spec = Spec(
        body=sq(a) * a + sq(b) * b * C2,
        reference=lambda in0, in1, s0, s1, imm2: (
            in0**3 + np.maximum(in0 - 1.0, 0.0) ** 3 * imm2
        ).astype(np.float32),
    )
    ops = (_register_dve_op("KAN_WIN_V1", win_spec),
           _register_dve_op("KAN_CUBE_V1", cube_spec))
    _DVE_OP_CACHE["ops"] = ops
    return ops


def _fold_coeff(coeff: np.ndarray):
    """coeff [COUT, CIN, KS, KS, NB] -> W_host [512, 576] bf16.

    Channels fed to the matmul are 6*phi_m(t), so the folded weights are
    coeff/6.  Row r = q*128 + (m%2)*64 + j (contraction); columns per the
    WCOLS layout above.
    """
    import ml_dtypes

    c = coeff.astype(np.float64) / 6.0  # [o, j, kk, l, m]
    w = np.zeros((NQ * 128, WCOLS), np.float64)
    for q in range(NQ):
        for half in range(2):
            m = 2 * q + half
            r = slice(q * 128 + half * 64, q * 128 + half * 64 + 64)
            sub = c[:, :, :, :, m]  # [o, j, kk, l]
            for kk in range(KS):
                w[r, kk * 128:kk * 128 + 64] = sub[:, :, kk, 0].T
                w[r, kk * 128 + 64:kk * 128 + 128] = sub[:, :, kk, 1].T
                w[r, 384 + kk * 128:384 + kk * 128 + 64] = sub[:, :, kk, 2].T
    return np.ascontiguousarray(w.astype(ml_dtypes.bfloat16))


def _groups():
    """[(g, y0, nr)] for one image."""
    return [(g, NRG * g, NRG if g < NG - 1 else HO - NRG * (NG - 1))
            for g in range(NG)]


def _build_bass():
    import concourse.bacc as bacc
    import concourse.mybir as mybir
    import concourse.tile as tile

    f32 = mybir.dt.float32
    bf16 = mybir.dt.bfloat16
    AF = mybir.ActivationFunctionType
    ALU = mybir.AluOpType
    kan_win, kan_cube = _get_kan_ops()

    nc = bacc.Bacc("TRN2", target_bir_lowering=False, debug=False,
                   num_devices=N_CORES)
    x_d = nc.dram_tensor("x", [B_SHARD, CIN, H, W], f32, kind="ExternalInput").ap()
    w_d = nc.dram_tensor("w", [NQ * 128, WCOLS], bf16, kind="ExternalInput").ap()
    out_d = nc.dram_tensor("out", [B_SHARD, COUT, HO, WO], f32,
                           kind="ExternalOutput").ap()

    strips_of = {0: STRIPS_IMG0, 1: STRIPS_IMG1}
    g2s_of = {0: G2S_IMG0, 1: G2S_IMG1}
    srow = {(b, s[0]): (s[1], s[2]) for b in (0, 1) for s in strips_of[b]}

    # which (strip, q) units run the window on Vector (KAN_WIN) instead of
    # Scalar (Abs+Relu): q3 everywhere, plus q1 for the tiny first strip.
    def on_vpath(sname, q):
        if sname == "s0a":
            return q in (0, 2)
        return q == 3

    with tile.TileContext(nc) as tc:
        from contextlib import ExitStack

        with ExitStack() as ctx:
            wpool = ctx.enter_context(tc.tile_pool(name="w", bufs=1))
            cpool = ctx.enter_context(tc.tile_pool(name="const", bufs=1))
            xpool = ctx.enter_context(tc.tile_pool(name="x", bufs=1))
            upool = ctx.enter_context(tc.tile_pool(name="u", bufs=2))
            apool = ctx.enter_context(tc.tile_pool(name="a", bufs=2))
            gpool = ctx.enter_context(tc.tile_pool(name="g", bufs=1))
            tpool = ctx.enter_context(tc.tile_pool(name="t", bufs=2))
            opool = ctx.enter_context(tc.tile_pool(name="o", bufs=2))
            obufs = {}
            ppool = ctx.enter_context(
                tc.tile_pool(name="ps", bufs=1, space="PSUM"))

            # ---- prologue DMAs ----
            # The sync HW queue drains first; the scalar queue is only
            # served after it.  Put everything on sync, in need-order.
            # per-partition constant table, built on-device (a DMA here
            # costs 128 tiny descriptors ~1.3us on the critical path):
            # cols 2q/2q+1: KAN_WIN s0=m+4, s1=m; cols 8+q: Abs bias
            # -(m+2); col 12: 2.0 (Relu bias).  m = 2q + (p>=64).
            bt = cpool.tile([128, 3 * NQ + 1], f32)
            for q in range(NQ):
                for half, p0 in ((0, 0), (1, 64)):
                    m = 2 * q + half
                    nc.vector.memset(bt[p0:p0 + 64, 2 * q:2 * q + 1],
                                     float(m + 4))
                    nc.vector.memset(bt[p0:p0 + 64, 2 * q + 1:2 * q + 2],
                                     float(m))
                    nc.vector.memset(bt[p0:p0 + 64,
                                        2 * NQ + q:2 * NQ + q + 1],
                                     float(-(m + 2)))
            nc.vector.memset(bt[:, 3 * NQ:3 * NQ + 1], 2.0)

            xts = {}  # (b, sname) -> x tile

            def emit_x(b, sname, eng, rsplit=None):
                y0, nrows = srow[(b, sname)]
                xt = xpool.tile([128, nrows * W], f32, tag=f"x_{sname}",
                                name=f"x{b}{sname}")
                src = x_d[b, :, y0:y0 + nrows, :]
                xv = xt[:].rearrange("p (r c) -> p r c", c=W)
                rows = ((0, nrows),) if rsplit is None else (
                    (0, rsplit), (rsplit, nrows))
                for r0, r1 in rows:
                    eng.dma_start(xv[0:64, r0:r1, :], src[:, r0:r1, :])
                    eng.dma_start(xv[64:128, r0:r1, :], src[:, r0:r1, :])
                xts[(b, sname)] = xt

            emit_x(0, "s0a", nc.sync)
            wts = []
            for q in range(NQ):
                wt = wpool.tile([128, WCOLS], bf16, tag=f"w{q}", name=f"w{q}")
                nc.sync.dma_start(wt[:], w_d[q * 128:(q + 1) * 128, :])
                wts.append(wt)
            emit_x(0, "s0b1", nc.sync)
            emit_x(0, "s0b2", nc.sync)
            emit_x(0, "s1a", nc.sync)
            emit_x(0, "s1b", nc.sync)
            emit_x(0, "s1c", nc.sync)
            emit_x(0, "s2", nc.sync)
            emit_x(1, "sA", nc.sync)
            emit_x(1, "sB", nc.sync)
            emit_x(1, "sC", nc.sync)

            # PE p-state pre-warm: dummy matmuls on scratch data keep the
            # PE busy from t~8us (engine-ready) so the clock is at 2.4GHz
            # when the first real matmul issues (~13us).  Outputs land in
            # the A0 bank and are overwritten by group 0's start=True.
            scr = gpool.tile([128, 1024], bf16, tag="g_scr", name="scr")
            nc.vector.memset(scr[:, 0:1], 0.0)
            pswarm = ppool.tile([128, NRG, 63], f32, tag="A0", name="warm")
            for _ in range(12):
                nc.tensor.matmul(pswarm[:, :, :],
                                 scr[:, 0:128], scr[:, 0:504],
                                 start=True, stop=True,
                                 skip_group_check=True)
            # fine-grained bridge: tiny dummies absorb xs0a arrival jitter
            # at ~30ns each instead of an idle gap that resets the p-state
            for _ in range(12):
                nc.tensor.matmul(pswarm[:, 0, :],
                                 scr[:, 0:128], scr[:, 0:63],
                                 start=True, stop=True,
                                 skip_group_check=True)

            gts = {}  # (b, sname, q) -> g tile (bf16)

            def emit_unit(b, sname, q):
                nrows = srow[(b, sname)][1]
                npx = nrows * W
                xt = xts[(b, sname)]
                a = apool.tile([128, UMAX], f32, tag="a", name="a")
                g = gpool.tile([128, npx], bf16, tag=f"g_{sname}_{q}",
                               name=f"g{sname}{q}")
                if b == 0 and sname == "s0a" and q == 0:
                    # 2 row-pieces: the first pair matmul (rows 0:8) can
                    # start as soon as piece 1 lands
                    for c0, c1 in ((0, 8 * W), (8 * W, npx)):
                        nc.vector._custom_dve(
                            kan_win, out=a[:, c0:c1], in0=xt[:, c0:c1],
                            s0=bt[:, 2 * q:2 * q + 1],
                            s1=bt[:, 2 * q + 1:2 * q + 2], imm2=11.0)
                        nc.vector._custom_dve(kan_cube, out=g[:, c0:c1],
                                              in0=a[:, c0:c1], imm2=-4.0)
                    gts[(b, sname, q)] = g
                    return
                if on_vpath(sname, q):
                    nc.vector._custom_dve(
                        kan_win, out=a[:, :npx], in0=xt[:],
                        s0=bt[:, 2 * q:2 * q + 1],
                        s1=bt[:, 2 * q + 1:2 * q + 2], imm2=11.0)
                else:
                    u = upool.tile([128, UMAX], f32, tag="u", name="u")
                    nc.scalar.activation(u[:, :npx], xt[:], AF.Abs,
                                         bias=bt[:, 2 * NQ + q:2 * NQ + q + 1],
                                         scale=11.0)
                    nc.scalar.activation(a[:, :npx], u[:, :npx], AF.Relu,
                                         bias=bt[:, 3 * NQ:3 * NQ + 1],
                                         scale=-1.0)
                nc.vector._custom_dve(kan_cube, out=g[:], in0=a[:, :npx],
                                      imm2=-4.0)
                gts[(b, sname, q)] = g

            def emit_U(b, sname):
                for q in range(NQ):
                    emit_unit(b, sname, q)

            groups = _groups()

            def emit_M(b, g, rr=None, tag=None):
                _, y0, nr = groups[g]
                sname = g2s_of[b][g]
                s0 = srow[(b, sname)][0]
                ro = y0 - s0
                if rr is not None:
                    y0, nr, ro = y0 + rr[0], rr[1] - rr[0], ro + rr[0]
                psA = ppool.tile([128, NRG, 63], f32, tag=tag or f"A{g % 8}",
                                 name=tag or f"A{g % 8}")
                gvs = [gts[(b, sname, q)][:].rearrange("p (r c) -> p r c",
                                                       c=W)
                       for q in range(NQ)]

                def pair(q, kk, start=False, stop=False):
                    nc.tensor.matmul(
                        psA[:, :nr, :],
                        wts[q][:, kk * 128:(kk + 1) * 128],
                        gvs[q][:, ro + kk:ro + kk + nr, 0:63],
                        start=start, stop=stop, skip_group_check=True)

                def single(q, kk):
                    nc.tensor.matmul(
                        psA[:, :nr, 0:62],
                        wts[q][:, 384 + kk * 128:384 + (kk + 1) * 128],
                        gvs[q][:, ro + kk:ro + kk + nr, 2:64],
                        start=False, stop=False, skip_group_check=True)

                # per-q order [pairs, singles]; q3 flips so the final
                # full-region pair closes the accumulation group (stop).
                for q in range(NQ):
                    if q == 0:
                        pair(0, 0, start=True)
                        pair(0, 1)
                        pair(0, 2)
                        for kk in range(KS):
                            single(0, kk)
                    elif q < NQ - 1:
                        for kk in range(KS):
                            pair(q, kk)
                        for kk in range(KS):
                            single(q, kk)
                    else:
                        for kk in range(KS):
                            single(q, kk)
                        pair(q, 0)
                        pair(q, 1)
                        pair(q, 2, stop=True)
                return (b, g, y0, nr, psA)

            def emit_C(mm):
                b, g, y0, nr, psA = mm
                if b not in obufs:
                    obufs[b] = opool.tile([64, HO * WO], f32, tag="ob",
                                          name=f"ob{b}")
                ov = obufs[b][:].rearrange("p (r c) -> p r c", c=WO)
                cA = tpool.tile([64, NRG, WO], f32, tag="cA", name="cA")
                nc.scalar.copy(cA[:, :nr, :], psA[0:64, :nr, 0:62])
                nc.vector.tensor_tensor(
                    ov[:, y0:y0 + nr, :], cA[:, :nr, :],
                    psA[64:128, :nr, 1:63], ALU.add)
                # two fat contiguous stores per image (the per-group store
                # pattern costs thousands of 248B DMA descriptors)
                if g in (3, 5, 6):
                    h0 = {3: 0, 5: 32, 6: 48}[g]
                    eng = nc.sync if g == 6 else nc.scalar
                    eng.dma_start(out_d[b, :, h0:y0 + nr, :],
                                  ov[:, h0:y0 + nr, :])
                elif g == 7 and b == 0:
                    nc.scalar.dma_start(out_d[b, :, 56:HO, :],
                                        ov[:, 56:HO, :])

            # ---- software-pipelined emission (unit granularity) ----
            SCHED = [
                ("U", 0, "s0a", 0), ("U", 0, "s0a", 1), ("U", 0, "s0a", 2),
                ("U", 0, "s0a", 3),
                ("U", 0, "s0b1", 0), ("U", 0, "s0b1", 1),
                ("U", 0, "s0b1", 2), ("U", 0, "s0b1", 3),
                ("M", 0, 0),
                ("U", 0, "s0b2", 0), ("U", 0, "s0b2", 1),
                ("U", 0, "s0b2", 2), ("U", 0, "s0b2", 3),
                ("M", 0, 1),
                ("U", 0, "s1a", 0), ("U", 0, "s1a", 1),
                ("U", 0, "s1a", 2), ("U", 0, "s1a", 3),
                ("M", 0, 2),
                ("U", 0, "s1b", 0), ("U", 0, "s1b", 1),
                ("U", 0, "s1b", 2), ("U", 0, "s1b", 3),
                ("M", 0, 3),
                ("U", 0, "s1c", 0), ("U", 0, "s1c", 1),
                ("U", 0, "s1c", 2), ("U", 0, "s1c", 3),
                ("M", 0, 4),
                ("U", 0, "s2", 0), ("U", 0, "s2", 1),
                ("U", 0, "s2", 2), ("U", 0, "s2", 3),
                ("M", 0, 5),
                ("U", 1, "sA", 0), ("U", 1, "sA", 1),
                ("M", 0, 6),
                ("U", 1, "sA", 2), ("U", 1, "sA", 3),
                ("M", 0, 7),
                ("U", 1, "sB", 0), ("U", 1, "sB", 1),
                ("M", 1, 0),
                ("U", 1, "sB", 2), ("U", 1, "sB", 3),
                ("M", 1, 1),
                ("U", 1, "sC", 0), ("U", 1, "sC", 1),
                ("M", 1, 2),
                ("U", 1, "sC", 2), ("U", 1, "sC", 3),
                ("M", 1, 3), ("M", 1, 4), ("M", 1, 5),
                ("M", 1, 6), ("M", 1, 7),
            ]
            def emit_C_last(mm):
                b, g, y0, nr, psA = mm
                ov = obufs[b][:].rearrange("p (r c) -> p r c", c=WO)
                h = nr // 2
                for r0, r1 in ((0, h), (h, nr)):
                    cA = tpool.tile([64, NRG, WO], f32, tag="cA", name="cA")
                    nc.scalar.copy(cA[:, :r1 - r0, :],
                                   psA[0:64, r0:r1, 0:62])
                    nc.vector.tensor_tensor(
                        ov[:, y0 + r0:y0 + r1, :], cA[:, :r1 - r0, :],
                        psA[64:128, r0:r1, 1:63], ALU.add)
                    eng = nc.sync if r0 == 0 else nc.scalar
                    eng.dma_start(out_d[b, :, y0 + r0:y0 + r1, :],
                                  ov[:, y0 + r0:y0 + r1, :])

            pending = []
            for blk in SCHED:
                if blk[0] == "U":
                    emit_unit(blk[1], blk[2], blk[3])
                    continue
                b, g = blk[1], blk[2]
                if b == 1 and g == NG - 1:
                    # final group: two half-groups on separate banks so the
                    # first half's combine overlaps the second half's
                    # matmuls, shortening the tail chain
                    nr = groups[g][2]
                    h = nr // 2
                    mmA = emit_M(b, g, rr=(0, h), tag="A7")
                    while pending:
                        emit_C(pending.pop(0))
                    mmB = emit_M(b, g, rr=(h, nr), tag="A6")
                    emit_C_last(mmA)
                    pending.append(mmB)
                    continue
                mm = emit_M(b, g)
                if pending:
                    emit_C(pending.pop(0))
                pending.append(mm)
            emit_C_last(pending.pop(0))

    nc.compile()
    return nc


def _maybe_install_profile_shim():
    """Allow trace=True/BASS_TRACE under axon even though this image lacks
    antenv.axon_hooks; degrade silently if anything is missing."""
    import types

    if "antenv.axon_hooks" in sys.modules:
        return
    try:
        from trn_agent_boot.trn_boot import _ntff_profile_via_ctypes

        hook = _ntff_profile_via_ctypes("/opt/axon/libaxon_pjrt.so")
        if hook is None:
            return
        mod = types.ModuleType("antenv.axon_hooks")
        mod.get_axon_ntff_profile_hook = lambda: hook
        mod.set_axon_ntff_profile_hook = lambda h: None
        sys.modules["antenv.axon_hooks"] = mod
        from concourse import bass_utils

        bass_utils.upload_artifacts = lambda tmpdir: f"local:{tmpdir}"
    except Exception:
        pass


_LAST_RESULTS = None


def kernel(x: np.ndarray, coeff: np.ndarray) -> np.ndarray:
    global _LAST_RESULTS
    from concourse import bass_utils

    _maybe_install_profile_shim()

    x = np.ascontiguousarray(np.asarray(x), dtype=np.float32)
    coeff = np.asarray(coeff)
    assert x.shape == (B_FULL, CIN, H, W), x.shape

    w_host = _fold_coeff(coeff)
    nc = _build_bass()

    in_maps = []
    for i in range(N_CORES):
        in_maps.append({
            "x": np.ascontiguousarray(x[i * B_SHARD:(i + 1) * B_SHARD]),
            "w": w_host,
        })

    res = bass_utils.run_bass_kernel_spmd(
        nc, in_maps, core_ids=list(range(N_CORES)),
        trace=bool(os.environ.get("KAN_TRACE")),
    )
    _LAST_RESULTS = res

    out = np.concatenate([res.results[i]["out"] for i in range(N_CORES)], axis=0)
    return out.astype(np.float32, copy=False)


# revision 33
# speedup vs baseline: 1.0100x; 1.0100x over previous
"""Trainium2 Bass kernel for the ConvolutionalKAN problem.

Math: the KAN conv
    out[b,o,y,x] = sum_{j,kk,l,m} phi_m(11*inp[b,j,y+kk,x+l]) * coeff[o,j,kk,l,m]
with phi_m the degree-3 B-spline basis on uniform knots linspace(0,1,12).
Uniform knots -> phi_m(t) = N3(t-m) with N3 the cardinal cubic B-spline:
    6*N3 = a^3 - 4*b^3,  a = relu(2-u), b = relu(1-u) = relu(a-1), u = |t-(m+2)|
Weights fold to coeff/6 exactly, making this a VALID 3x3 conv over
64*8 = 512 input channels.

v3 design (vs the 146us v2, which was PE-bound at 50% column utilization):
- Tap pairing: the 6 taps with l in {0,1} ride 128-wide lhsTs = two taps
  ((kk,0)|(kk,1)) x 64 Cout sharing one [nr,63] rhs stream, so all 128 PE
  output columns are used for 2/3 of the work.  The 3 l=2 taps stay
  64-wide but fold into the SAME psum bank: their shifted [nr,62] windows
  plus a strided out-AP land them exactly on the anchor (l=0) alignment.
  One accumulation bank per 8-row output group:
      psA[0:64,  dy, x]   = sum of taps (kk,0) and (kk,2) partials
      psA[64:128, dy, x+1] = sum of taps (kk,1) partials
  PE cycles drop from ~287k to ~186k per core; the combine is only
  1 ACT copy (Scalar, psum->sbuf) + 1 shifted add (Vector, sbuf+psum)
  per group.
- Basis eval is split: most (q,strip) units compute the spline window
  a = relu(2-|11x-(m+2)|) on the Scalar engine (Abs+Relu ACTs); q3 units
  use the custom DVE op KAN_WIN on Vector.  The cube
  g = a^3 - 4*relu(a-1)^3 always runs on Vector (custom DVE op KAN_CUBE,
  bf16 out).
- Startup: weights ride the sync HW-DGE queue immediately; image 0's
  first 10 input rows form a tiny sub-strip so the first matmul chain
  starts early.  All 8 psum banks cycle through the 8-group pipeline.

Sharding: data-parallel over batch, 2 images per core on 8 cores.
"""

import os
import sys

import numpy as np

for _p in ("/root/.axon_site/_ro/trn_rl_repo", "/opt/trn_rl_repo"):
    if os.path.isdir(_p) and _p not in sys.path:
        sys.path.append(_p)

B_FULL = 16
N_CORES = 8
B_SHARD = B_FULL // N_CORES
CIN = 64
COUT = 64
H = 64
W = 64
KS = 3
NB = 8
NS = 8
HO = H - KS + 1  # 62
WO = W - KS + 1  # 62
NQ = (CIN * NS) // 128  # 4 contraction tiles of 128
NRG = 8  # output rows per group
NG = 8  # groups per image (7x8 + 1x6)

# weight column layout (per 128-row contraction chunk):
#   cols kk*128+[0:64]  = tap (kk,0), kk*128+[64:128] = tap (kk,1)
#   cols 384+kk*128+[0:64] = tap (kk,2), [64:128] zero (pad keeps every
#   LDWEIGHTS 128 wide; the zero half accumulates 0 into psA[64:128])
WCOLS = 768

# strips: (name, first input row, n rows); group g (out rows 8g..8g+nr-1)
# reads input rows 8g..8g+nr+1.  Image 0 splits the first strip so group
# 0's basis is ready early.
STRIPS_IMG0 = (("s0a", 0, 10), ("s0b1", 8, 10), ("s0b2", 16, 10),
               ("s1a", 24, 10), ("s1b", 32, 10), ("s1c", 40, 10),
               ("s2", 48, 16))
G2S_IMG0 = ("s0a", "s0b1", "s0b2", "s1a", "s1b", "s1c", "s2", "s2")
STRIPS_IMG1 = (("sA", 0, 26), ("sB", 24, 26), ("sC", 48, 16))
G2S_IMG1 = ("sA", "sA", "sA", "sB", "sB", "sB", "sC", "sC")

UMAX = 26 * W  # largest strip, scratch tile size

_DVE_OP_CACHE = {}


def _register_dve_op(name, spec):
    from concourse import dve_ops
    from concourse.dve_spec import lower
    from concourse.dve_uop import DveOpSpec
    from concourse.dve_spec import _has_src1

    existing = {op.name for op in dve_ops.OPS}
    if name in existing:
        return next(o for o in dve_ops.OPS if o.name == name)
    row = dve_ops._CUSTOM_DVE_ROW_BASE + len(dve_ops.OPS)
    shas = {}
    for ver in ("v3", "v4"):
        s = DveOpSpec(name=name, opcode=row, uops=lower(spec, ver=ver),
                      rd1_en=_has_src1(spec))
        shas[ver] = s.sha(ver)
    op = dve_ops.DveOp(name, spec, subdim=False, uops_sha=shas)
    dve_ops.OPS.append(op)
    dve_ops._SUB_OPCODE_FOR_NAME[name] = row
    return op


def _get_kan_ops():
    """Register (once) and return the two custom DVE ops:
    KAN_WIN:  a = relu(min(s0 - 11*x, 11*x - s1))  (= relu(2-|11x-(m+2)|)
              for s0 = m+4, s1 = m)
    KAN_CUBE: g = a^3 - 4*relu(a-1)^3              (= 6*N3(|11x-(m+2)|))
    """
    if "ops" in _DVE_OP_CACHE:
        return _DVE_OP_CACHE["ops"]
    from concourse.dve_spec import C0, C1, C2, One, Spec, Src0, minn, relu, sq

    m = Src0 * C2
    win_spec = Spec(
        body=relu(minn(C0 - m, m - C1)),
        reference=lambda in0, in1, s0, s1, imm2: np.maximum(
            np.minimum(s0 - in0 * imm2, in0 * imm2 - s1), 0.0
        ).astype(np.float32),
    )
    a = Src0
    b = relu(a - One)
    cube_spec = Spec(
        body=sq(a) * a + sq(b) * b * C2,
        reference=lambda in0, in1, s0, s1, imm2: (
            in0**3 + np.maximum(in0 - 1.0, 0.0) ** 3 * imm2
        ).astype(np.float32),
    )
    ops = (_register_dve_op("KAN_WIN_V1", win_spec),
           _register_dve_op("KAN_CUBE_V1", cube_spec))
    _DVE_OP_CACHE["ops"] = ops
    return ops


def _fold_coeff(coeff: np.ndarray):
    """coeff [COUT, CIN, KS, KS, NB] -> W_host [512, 576] bf16.

    Channels fed to the matmul are 6*phi_m(t), so the folded weights are
    coeff/6.  Row r = q*128 + (m%2)*64 + j (contraction); columns per the
    WCOLS layout above.
    """
    import ml_dtypes

    c = coeff.astype(np.float64) / 6.0  # [o, j, kk, l, m]
    w = np.zeros((NQ * 128, WCOLS), np.float64)
    for q in range(NQ):
        for half in range(2):
            m = 2 * q + half
            r = slice(q * 128 + half * 64, q * 128 + half * 64 + 64)
            sub = c[:, :, :, :, m]  # [o, j, kk, l]
            for kk in range(KS):
                w[r, kk * 128:kk * 128 + 64] = sub[:, :, kk, 0].T
                w[r, kk * 128 + 64:kk * 128 + 128] = sub[:, :, kk, 1].T
                w[r, 384 + kk * 128:384 + kk * 128 + 64] = sub[:, :, kk, 2].T
    return np.ascontiguousarray(w.astype(ml_dtypes.bfloat16))


def _groups():
    """[(g, y0, nr)] for one image."""
    return [(g, NRG * g, NRG if g < NG - 1 else HO - NRG * (NG - 1))
            for g in range(NG)]


def _build_bass():
    import concourse.bacc as bacc
    import concourse.mybir as mybir
    import concourse.tile as tile

    f32 = mybir.dt.float32
    bf16 = mybir.dt.bfloat16
    AF = mybir.ActivationFunctionType
    ALU = mybir.AluOpType
    kan_win, kan_cube = _get_kan_ops()

    nc = bacc.Bacc("TRN2", target_bir_lowering=False, debug=False,
                   num_devices=N_CORES)
    x_d = nc.dram_tensor("x", [B_SHARD, CIN, H, W], f32, kind="ExternalInput").ap()
    w_d = nc.dram_tensor("w", [NQ * 128, WCOLS], bf16, kind="ExternalInput").ap()
    out_d = nc.dram_tensor("out", [B_SHARD, COUT, HO, WO], f32,
                           kind="ExternalOutput").ap()

    strips_of = {0: STRIPS_IMG0, 1: STRIPS_IMG1}
    g2s_of = {0: G2S_IMG0, 1: G2S_IMG1}
    srow = {(b, s[0]): (s[1], s[2]) for b in (0, 1) for s in strips_of[b]}

    # which (strip, q) units run the window on Vector (KAN_WIN) instead of
    # Scalar (Abs+Relu): q3 everywhere, plus q1 for the tiny first strip.
    def on_vpath(sname, q):
        if sname == "s0a":
            return q in (0, 2)
        return q == 3

    with tile.TileContext(nc) as tc:
        from contextlib import ExitStack

        with ExitStack() as ctx:
            wpool = ctx.enter_context(tc.tile_pool(name="w", bufs=1))
            cpool = ctx.enter_context(tc.tile_pool(name="const", bufs=1))
            xpool = ctx.enter_context(tc.tile_pool(name="x", bufs=1))
            upool = ctx.enter_context(tc.tile_pool(name="u", bufs=2))
            apool = ctx.enter_context(tc.tile_pool(name="a", bufs=2))
            gpool = ctx.enter_context(tc.tile_pool(name="g", bufs=1))
            tpool = ctx.enter_context(tc.tile_pool(name="t", bufs=2))
            opool = ctx.enter_context(tc.tile_pool(name="o", bufs=2))
            obufs = {}
            ppool = ctx.enter_context(
                tc.tile_pool(name="ps", bufs=1, space="PSUM"))

            # ---- prologue DMAs ----
            # The sync HW queue drains first; the scalar queue is only
            # served after it.  Put everything on sync, in need-order.
            # per-partition constant table, built on-device (a DMA here
            # costs 128 tiny descriptors ~1.3us on the critical path):
            # cols 2q/2q+1: KAN_WIN s0=m+4, s1=m; cols 8+q: Abs bias
            # -(m+2); col 12: 2.0 (Relu bias).  m = 2q + (p>=64).
            bt = cpool.tile([128, 3 * NQ + 1], f32)
            for q in range(NQ):
                for half, p0 in ((0, 0), (1, 64)):
                    m = 2 * q + half
                    nc.vector.memset(bt[p0:p0 + 64, 2 * q:2 * q + 1],
                                     float(m + 4))
                    nc.vector.memset(bt[p0:p0 + 64, 2 * q + 1:2 * q + 2],
                                     float(m))
                    nc.vector.memset(bt[p0:p0 + 64,
                                        2 * NQ + q:2 * NQ + q + 1],
                                     float(-(m + 2)))
            nc.vector.memset(bt[:, 3 * NQ:3 * NQ + 1], 2.0)

            xts = {}  # (b, sname) -> x tile

            def emit_x(b, sname, eng, rsplit=None):
                y0, nrows = srow[(b, sname)]
                xt = xpool.tile([128, nrows * W], f32, tag=f"x_{sname}",
                                name=f"x{b}{sname}")
                src = x_d[b, :, y0:y0 + nrows, :]
                xv = xt[:].rearrange("p (r c) -> p r c", c=W)
                rows = ((0, nrows),) if rsplit is None else (
                    (0, rsplit), (rsplit, nrows))
                for r0, r1 in rows:
                    eng.dma_start(xv[0:64, r0:r1, :], src[:, r0:r1, :])
                    eng.dma_start(xv[64:128, r0:r1, :], src[:, r0:r1, :])
                xts[(b, sname)] = xt

            emit_x(0, "s0a", nc.sync)
            wts = []
            for q in range(NQ):
                wt = wpool.tile([128, WCOLS], bf16, tag=f"w{q}", name=f"w{q}")
                nc.sync.dma_start(wt[:], w_d[q * 128:(q + 1) * 128, :])
                wts.append(wt)
            emit_x(0, "s0b1", nc.sync)
            emit_x(0, "s0b2", nc.sync)
            emit_x(0, "s1a", nc.sync)
            emit_x(0, "s1b", nc.sync)
            emit_x(0, "s1c", nc.sync)
            emit_x(0, "s2", nc.sync)
            emit_x(1, "sA", nc.sync)
            emit_x(1, "sB", nc.sync)
            emit_x(1, "sC", nc.sync)

            # PE p-state pre-warm: dummy matmuls on scratch data keep the
            # PE busy from t~8us (engine-ready) so the clock is at 2.4GHz
            # when the first real matmul issues (~13us).  Outputs land in
            # the A0 bank and are overwritten by group 0's start=True.
            scr = gpool.tile([128, 1024], bf16, tag="g_scr", name="scr")
            nc.vector.memset(scr[:, 0:1], 0.0)
            pswarm = ppool.tile([128, NRG, 64], f32, tag="A0", name="warm")
            for _ in range(13):
                nc.tensor.matmul(pswarm[:, :, :],
                                 scr[:, 0:128], scr[:, 0:512],
                                 start=True, stop=True,
                                 skip_group_check=True)

            gts = {}  # (b, sname, q) -> g tile (bf16)

            def emit_unit(b, sname, q):
                nrows = srow[(b, sname)][1]
                npx = nrows * W
                xt = xts[(b, sname)]
                a = apool.tile([128, UMAX], f32, tag="a", name="a")
                g = gpool.tile([128, npx], bf16, tag=f"g_{sname}_{q}",
                               name=f"g{sname}{q}")
                if b == 0 and sname == "s0a" and q == 0:
                    # 2 row-pieces: the first pair matmul (rows 0:8) can
                    # start as soon as piece 1 lands
                    for c0, c1 in ((0, 8 * W), (8 * W, npx)):
                        nc.vector._custom_dve(
                            kan_win, out=a[:, c0:c1], in0=xt[:, c0:c1],
                            s0=bt[:, 2 * q:2 * q + 1],
                            s1=bt[:, 2 * q + 1:2 * q + 2], imm2=11.0)
                        nc.vector._custom_dve(kan_cube, out=g[:, c0:c1],
                                              in0=a[:, c0:c1], imm2=-4.0)
                    gts[(b, sname, q)] = g
                    return
                if on_vpath(sname, q):
                    nc.vector._custom_dve(
                        kan_win, out=a[:, :npx], in0=xt[:],
                        s0=bt[:, 2 * q:2 * q + 1],
                        s1=bt[:, 2 * q + 1:2 * q + 2], imm2=11.0)
                else:
                    u = upool.tile([128, UMAX], f32, tag="u", name="u")
                    nc.scalar.activation(u[:, :npx], xt[:], AF.Abs,
                                         bias=bt[:, 2 * NQ + q:2 * NQ + q + 1],
                                         scale=11.0)
                    nc.scalar.activation(a[:, :npx], u[:, :npx], AF.Relu,
                                         bias=bt[:, 3 * NQ:3 * NQ + 1],
                                         scale=-1.0)
                nc.vector._custom_dve(kan_cube, out=g[:], in0=a[:, :npx],
                                      imm2=-4.0)
                gts[(b, sname, q)] = g

            def emit_U(b, sname):
                for q in range(NQ):
                    emit_unit(b, sname, q)

            groups = _groups()

            def emit_M(b, g, rr=None, tag=None):
                _, y0, nr = groups[g]
                sname = g2s_of[b][g]
                s0 = srow[(b, sname)][0]
                ro = y0 - s0
                if rr is not None:
                    y0, nr, ro = y0 + rr[0], rr[1] - rr[0], ro + rr[0]
                psA = ppool.tile([128, NRG, 64], f32, tag=tag or f"A{g % 8}",
                                 name=tag or f"A{g % 8}")
                gfs = [gts[(b, sname, q)][:] for q in range(NQ)]
                gvs = [gf.rearrange("p (r c) -> p r c", c=W) for gf in gfs]

                def pair(q, kk, start=False, stop=False):
                    # full-width contiguous stream (no per-row AP jumps);
                    # cols 62/63 of each psum row are never-read garbage
                    nc.tensor.matmul(
                        psA[:, :nr, :],
                        wts[q][:, kk * 128:(kk + 1) * 128],
                        gfs[q][:, (ro + kk) * W:(ro + kk + nr) * W],
                        start=start, stop=stop, skip_group_check=True)

                def single(q, kk):
                    nc.tensor.matmul(
                        psA[:, :nr, 0:62],
                        wts[q][:, 384 + kk * 128:384 + (kk + 1) * 128],
                        gvs[q][:, ro + kk:ro + kk + nr, 2:64],
                        start=False, stop=False, skip_group_check=True)

                # per-q order [pairs, singles]; q3 flips so the final
                # full-region pair closes the accumulation group (stop).
                for q in range(NQ):
                    if q == 0:
                        pair(0, 0, start=True)
                        pair(0, 1)
                        pair(0, 2)
                        for kk in range(KS):
                            single(0, kk)
                    elif q < NQ - 1:
                        for kk in range(KS):
                            pair(q, kk)
                        for kk in range(KS):
                            single(q, kk)
                    else:
                        for kk in range(KS):
                            single(q, kk)
                        pair(q, 0)
                        pair(q, 1)
                        pair(q, 2, stop=True)
                return (b, g, y0, nr, psA)

            def emit_C(mm):
                b, g, y0, nr, psA = mm
                if b not in obufs:
                    obufs[b] = opool.tile([64, HO * WO], f32, tag="ob",
                                          name=f"ob{b}")
                ov = obufs[b][:].rearrange("p (r c) -> p r c", c=WO)
                cA = tpool.tile([64, NRG, WO], f32, tag="cA", name="cA")
                nc.scalar.copy(cA[:, :nr, :], psA[0:64, :nr, 0:62])
                nc.vector.tensor_tensor(
                    ov[:, y0:y0 + nr, :], cA[:, :nr, :],
                    psA[64:128, :nr, 1:63], ALU.add)
                # two fat contiguous stores per image (the per-group store
                # pattern costs thousands of 248B DMA descriptors)
                if g in (3, 5, 6):
                    h0 = {3: 0, 5: 32, 6: 48}[g]
                    eng = nc.sync if g == 6 else nc.scalar
                    eng.dma_start(out_d[b, :, h0:y0 + nr, :],
                                  ov[:, h0:y0 + nr, :])
                elif g == 7 and b == 0:
                    nc.scalar.dma_start(out_d[b, :, 56:HO, :],
                                        ov[:, 56:HO, :])

            # ---- software-pipelined emission (unit granularity) ----
            SCHED = [
                ("U", 0, "s0a", 0), ("U", 0, "s0a", 1), ("U", 0, "s0a", 2),
                ("U", 0, "s0a", 3),
                ("U", 0, "s0b1", 0), ("U", 0, "s0b1", 1),
                ("U", 0, "s0b1", 2), ("U", 0, "s0b1", 3),
                ("M", 0, 0),
                ("U", 0, "s0b2", 0), ("U", 0, "s0b2", 1),
                ("U", 0, "s0b2", 2), ("U", 0, "s0b2", 3),
                ("M", 0, 1),
                ("U", 0, "s1a", 0), ("U", 0, "s1a", 1),
                ("U", 0, "s1a", 2), ("U", 0, "s1a", 3),
                ("M", 0, 2),
                ("U", 0, "s1b", 0), ("U", 0, "s1b", 1),
                ("U", 0, "s1b", 2), ("U", 0, "s1b", 3),
                ("M", 0, 3),
                ("U", 0, "s1c", 0), ("U", 0, "s1c", 1),
                ("U", 0, "s1c", 2), ("U", 0, "s1c", 3),
                ("M", 0, 4),
                ("U", 0, "s2", 0), ("U", 0, "s2", 1),
                ("U", 0, "s2", 2), ("U", 0, "s2", 3),
                ("M", 0, 5),
                ("U", 1, "sA", 0), ("U", 1, "sA", 1),
                ("M", 0, 6),
                ("U", 1, "sA", 2), ("U", 1, "sA", 3),
                ("M", 0, 7),
                ("U", 1, "sB", 0), ("U", 1, "sB", 1),
                ("M", 1, 0),
                ("U", 1, "sB", 2), ("U", 1, "sB", 3),
                ("M", 1, 1),
                ("U", 1, "sC", 0), ("U", 1, "sC", 1),
                ("M", 1, 2),
                ("U", 1, "sC", 2), ("U", 1, "sC", 3),
                ("M", 1, 3), ("M", 1, 4), ("M", 1, 5),
                ("M", 1, 6), ("M", 1, 7),
            ]
            def emit_C_last(mm):
                b, g, y0, nr, psA = mm
                ov = obufs[b][:].rearrange("p (r c) -> p r c", c=WO)
                h = nr // 2
                for r0, r1 in ((0, h), (h, nr)):
                    cA = tpool.tile([64, NRG, WO], f32, tag="cA", name="cA")
                    nc.scalar.copy(cA[:, :r1 - r0, :],
                                   psA[0:64, r0:r1, 0:62])
                    nc.vector.tensor_tensor(
                        ov[:, y0 + r0:y0 + r1, :], cA[:, :r1 - r0, :],
                        psA[64:128, r0:r1, 1:63], ALU.add)
                    eng = nc.sync if r0 == 0 else nc.scalar
                    eng.dma_start(out_d[b, :, y0 + r0:y0 + r1, :],
                                  ov[:, y0 + r0:y0 + r1, :])

            pending = []
            for blk in SCHED:
                if blk[0] == "U":
                    emit_unit(blk[1], blk[2], blk[3])
                    continue
                b, g = blk[1], blk[2]
                if b == 1 and g == NG - 1:
                    # final group: two half-groups on separate banks so the
                    # first half's combine overlaps the second half's
                    # matmuls, shortening the tail chain
                    nr = groups[g][2]
                    h = nr // 2
                    mmA = emit_M(b, g, rr=(0, h), tag="A7")
                    while pending:
                        emit_C(pending.pop(0))
                    mmB = emit_M(b, g, rr=(h, nr), tag="A6")
                    emit_C_last(mmA)
                    pending.append(mmB)
                    continue
                mm = emit_M(b, g)
                if pending:
                    emit_C(pending.pop(0))
                pending.append(mm)
            emit_C_last(pending.pop(0))

    nc.compile()
    return nc


def _maybe_install_profile_shim():
    """Allow trace=True/BASS_TRACE under axon even though this image lacks
    antenv.axon_hooks; degrade silently if anything is missing."""
    import types

    if "antenv.axon_hooks" in sys.modules:
        return
    try:
        from trn_agent_boot.trn_boot import _ntff_profile_via_ctypes

        hook = _ntff_profile_via_ctypes("/opt/axon/libaxon_pjrt.so")
        if hook is None:
            return
        mod = types.ModuleType("antenv.axon_hooks")
        mod.get_axon_ntff_profile_hook = lambda: hook
        mod.set_axon_ntff_profile_hook = lambda h: None
        sys.modules["antenv.axon_hooks"] = mod
        from concourse import bass_utils

        bass_utils.upload_artifacts = lambda tmpdir: f"local:{tmpdir}"
    except Exception:
        pass


_LAST_RESULTS = None


def kernel(x: np.ndarray, coeff: np.ndarray) -> np.ndarray:
    global _LAST_RESULTS
    from concourse import bass_utils

    _maybe_install_profile_shim()

    x = np.ascontiguousarray(np.asarray(x), dtype=np.float32)
    coeff = np.asarray(coeff)
    assert x.shape == (B_FULL, CIN, H, W), x.shape

    w_host = _fold_coeff(coeff)
    nc = _build_bass()

    in_maps = []
    for i in range(N_CORES):
        in_maps.append({
            "x": np.ascontiguousarray(x[i * B_SHARD:(i + 1) * B_SHARD]),
            "w": w_host,
        })

    res = bass_utils.run_bass_kernel_spmd(
        nc, in_maps, core_ids=list(range(N_CORES)),
        trace=bool(os.environ.get("KAN_TRACE")),
    )
    _LAST_RESULTS = res

    out = np.concatenate([res.results[i]["out"] for i in range(N_CORES)], axis=0)
    return out.astype(np.float32, copy=False)
